# revision 17
# baseline (speedup 1.0000x reference)
"""DGCNN-style graph conv kernel for Trainium2 (8 NeuronCores, data-parallel over batch).

Reference computation (per sample):
  idx = knn(xyz, 20)                        # top-20 by -||xi-xj||^2, per point
  geo = relu(BN1(w1 @ [nb_xyz - xyz; xyz]))
  fea = relu(BN2(w2 @ [nb_feat - feat; feat]))
  out = max_k concat([geo, fea])            # (128, N)

Algebraic collapse used here (relu/max commute, BN scale > 0):
  out[c, n] = relu( max_k G[c, idx[n, k]] + H[c, n] + hb[c] )
  G = s * (Wa @ X)          (neighbor part, gathered)
  H = s * ((Wb - Wa) @ X)   (center part)
  hb = s * b + shift        (folded BN bias)
where for c < 64: Wa/Wb from w1, X = xyz; for c >= 64: from w2, X = feat.

Neighbor 0 is always the point itself (self-distance is the unique maximum of
-d^2), so only 19 indices per point are gathered; the self term G[:, n] is
folded in with a plain elementwise max.

Device pipeline per core (1 sample):
  1. D-chunk (128 rows x 2048) = -(dist^2) via one K=5 augmented fp32 matmul:
     lhsT = [xyz; xx; 1], rhs = [2*xyz; -1; -xx]
  2. top-20 per row: 3 rounds of (max8, max_index8, match_replace8) on PSUM
  3. indices 1..19 -> DRAM in a 16-wrapped layout, reloaded for ap_gather
  4. ap_gather columns of G (SBUF), tensor_reduce max over k; the reduce for
     gather group b is emitted AFTER the top-k of group b+1 so the Vector
     engine never stalls behind an in-flight gather.
"""
import numpy as np

B, N, C, K = 8, 2048, 128, 20
KG = K - 1           # 19 gathered neighbors (self handled separately)
H2 = C // 2          # 64
EPS = 1e-5
NEG = -3.0e38
NCHUNK = N // 128    # 16 topk chunks
# gather groups in chunks: small first groups so the (serial, dominant)
# GpSimd gather stream starts right after chunk 0's top-k
GROUPS = [(0, 1), (1, 2), (2, 4), (4, 8), (8, 12), (12, 16)]
NI_CH = 128 * KG     # 2432 indices per chunk
NI_MAX = 4 * NI_CH   # largest gather (4 chunks)

_compiled = None


def _build():
    import concourse.bass as bass
    import concourse.bacc as bacc
    import concourse.mybir as mybir
    import concourse.tile as tile
    from concourse import library_config

    f32 = mybir.dt.float32
    u16 = mybir.dt.uint16

    nc = bacc.Bacc("TRN2")
    xyz_in = nc.declare_dram_parameter("xyz", [3, N], f32, isOutput=False)
    feat_in = nc.declare_dram_parameter("feat", [C, N], f32, isOutput=False)
    wg_xyz_in = nc.declare_dram_parameter("wg_xyz", [3, H2], f32, isOutput=False)
    wg_feat_in = nc.declare_dram_parameter("wg_feat", [C, H2], f32, isOutput=False)
    wh_xyz_in = nc.declare_dram_parameter("wh_xyz", [3, H2], f32, isOutput=False)
    wh_feat_in = nc.declare_dram_parameter("wh_feat", [C, H2], f32, isOutput=False)
    hb_in = nc.declare_dram_parameter("hb", [C, 1], f32, isOutput=False)
    out_dram = nc.declare_dram_parameter("out", [C, N], f32, isOutput=True)

    # wrapped index scratch, replicated across the 8 core groups at write
    # time so each gather needs only ONE reload DMA (one completion sem for
    # GpSimd to check): row (g16 p16), col (ch*152 + ph*19 + q)
    idxw_dram = nc.dram_tensor("idxw_scratch", [128, N * KG // 16], u16)

    with tile.TileContext(nc) as tc:
        with (
            tc.tile_pool(name="const", bufs=1) as cpool,
            tc.tile_pool(name="work", bufs=2) as wpool,
            tc.tile_pool(name="ag", bufs=2) as agpool,
            tc.tile_pool(name="psum", bufs=2, space="PSUM") as ppool,
        ):
            nc.gpsimd.load_library(library_config.ap_gather)

            xyz_t = cpool.tile([3, N], f32)
            feat_t = cpool.tile([C, N], f32)
            wgx_t = cpool.tile([3, H2], f32)
            wgf_t = cpool.tile([C, H2], f32)
            whx_t = cpool.tile([3, H2], f32)
            whf_t = cpool.tile([C, H2], f32)
            hb_t = cpool.tile([C, 1], f32)
            nc.sync.dma_start(xyz_t[:], xyz_in[:])
            nc.sync.dma_start(feat_t[:], feat_in[:])
            nc.sync.dma_start(wgx_t[:], wg_xyz_in[:])
            nc.sync.dma_start(wgf_t[:], wg_feat_in[:])
            nc.sync.dma_start(whx_t[:], wh_xyz_in[:])
            nc.sync.dma_start(whf_t[:], wh_feat_in[:])
            nc.sync.dma_start(hb_t[:], hb_in[:])

            # ---- xx[n] = sum_d xyz[d,n]^2 ----
            sq_t = cpool.tile([3, N], f32)
            nc.vector.tensor_tensor(
                out=sq_t[:], in0=xyz_t[:], in1=xyz_t[:], op=mybir.AluOpType.mult
            )
            ones3_t = cpool.tile([3, 1], f32)
            nc.vector.memset(ones3_t[:], 1.0)
            xx_ps = ppool.tile([1, N], f32, space="PSUM", tag="d")
            for j in range(4):
                nc.tensor.matmul(
                    out=xx_ps[:, 512 * j:512 * (j + 1)],
                    lhsT=ones3_t[:],
                    rhs=sq_t[:, 512 * j:512 * (j + 1)],
                    start=True, stop=True,
                )
            xx_t = cpool.tile([1, N], f32)
            nc.scalar.copy(xx_t[:], xx_ps[:])

            # ---- lhs5 = [xyz; xx; 1], rhs5 = [2 xyz; -1; -xx] ----
            # compute-engine ops must start at quadrant-aligned partitions, so
            # rows 3/4 are placed with SBUF->SBUF DMAs instead.
            lhs5 = cpool.tile([5, N], f32)
            rhs5 = cpool.tile([5, N], f32)
            ones_row = cpool.tile([1, N], f32)
            neg1_row = cpool.tile([1, N], f32)
            nxx_t = cpool.tile([1, N], f32)
            nc.vector.memset(ones_row[:], 1.0)
            nc.vector.memset(neg1_row[:], -1.0)
            nc.vector.tensor_scalar_mul(nxx_t[:], xx_t[:], -1.0)
            nc.vector.tensor_copy(lhs5[0:3, :], xyz_t[:])
            nc.vector.tensor_scalar_mul(rhs5[0:3, :], xyz_t[:], 2.0)
            nc.sync.dma_start(lhs5[3:4, :], xx_t[:])
            nc.sync.dma_start(lhs5[4:5, :], ones_row[:])
            nc.sync.dma_start(rhs5[3:4, :], neg1_row[:])
            nc.sync.dma_start(rhs5[4:5, :], nxx_t[:])

            # ---- G, H (128, N) ----
            g_ps = ppool.tile([C, N], f32, space="PSUM", tag="d")
            for j in range(4):
                fs = slice(512 * j, 512 * (j + 1))
                nc.tensor.matmul(out=g_ps[0:H2, fs], lhsT=wgx_t[:], rhs=xyz_t[:, fs],
                                 start=True, stop=True)
            for j in range(4):
                fs = slice(512 * j, 512 * (j + 1))
                nc.tensor.matmul(out=g_ps[H2:C, fs], lhsT=wgf_t[:], rhs=feat_t[:, fs],
                                 start=True, stop=True)
            g_t = cpool.tile([C, N], f32)
            nc.scalar.copy(g_t[:], g_ps[:])

            h_ps = ppool.tile([C, N], f32, space="PSUM", tag="d")
            for j in range(4):
                fs = slice(512 * j, 512 * (j + 1))
                nc.tensor.matmul(out=h_ps[0:H2, fs], lhsT=whx_t[:], rhs=xyz_t[:, fs],
                                 start=True, stop=True)
            for j in range(4):
                fs = slice(512 * j, 512 * (j + 1))
                nc.tensor.matmul(out=h_ps[H2:C, fs], lhsT=whf_t[:], rhs=feat_t[:, fs],
                                 start=True, stop=True)
            h_t = cpool.tile([C, N], f32)
            nc.scalar.copy(h_t[:], h_ps[:])

            # wrapped idx write view: (128, NCHUNK*8*KG) -> [g8, ch, ph, p16, q]
            idxw_w = idxw_dram[:].rearrange(
                "(g p) (ch ph q) -> g ch ph p q", g=8, ch=NCHUNK, ph=8, q=KG
            )

            # ---- per-chunk: D matmul + top-20 ----
            def emit_chunk(c):
                d_ps = ppool.tile([128, N], f32, space="PSUM", tag="d")
                for j in range(4):
                    fs = slice(512 * j, 512 * (j + 1))
                    nc.tensor.matmul(
                        out=d_ps[:, fs],
                        lhsT=lhs5[:, 128 * c:128 * (c + 1)],
                        rhs=rhs5[:, fs],
                        start=True, stop=True,
                    )
                d_sb = wpool.tile([128, N], f32, tag="dsb")
                nc.scalar.copy(d_sb[:], d_ps[:])
                vals = wpool.tile([128, 24], f32, tag="vals")
                idxs = wpool.tile([128, 24], u16, tag="idxs")
                for r in range(3):
                    v8 = vals[:, 8 * r:8 * (r + 1)]
                    i8 = idxs[:, 8 * r:8 * (r + 1)]
                    nc.vector.max(out=v8, in_=d_sb[:])
                    nc.vector.max_index(out=i8, in_max=v8, in_values=d_sb[:])
                    if r < 2:
                        nc.vector.match_replace(
                            out=d_sb[:], in_to_replace=v8, in_values=d_sb[:],
                            imm_value=NEG,
                        )
                # write top 1..19 indices (skip self at slot 0), replicated
                # into each core group's 16 rows
                for g in range(8):
                    nc.sync.dma_start(idxw_w[g, c], idxs[:, 1:K])

            # ---- gather start: idx reload + ap_gather (GpSimd + DMA only) ----
            ag_tiles = {}
            out_tiles = {}

            def emit_gather_start(b):
                c0, c1 = GROUPS[b]
                ni = (c1 - c0) * NI_CH
                idxw_t = agpool.tile([128, NI_MAX // 16], u16, tag="idxw")
                nc.sync.dma_start(
                    idxw_t[:, 0:ni // 16],
                    idxw_dram[:, (NI_CH // 16) * c0:(NI_CH // 16) * c1],
                )
                ag = agpool.tile([128, NI_MAX], f32, tag="ag")
                nc.gpsimd.ap_gather(
                    out_ap=ag[:, 0:ni],
                    in_ap=g_t[:],
                    idxs_ap=idxw_t[:, 0:ni // 16].bitcast(mybir.dt.int16),
                    channels=128, num_elems=N, d=1, num_idxs=ni,
                )
                ag_tiles[b] = ag

            # ---- gather finish: reduce + self-max + bias + relu (Vector) ----
            def emit_gather_finish(b):
                c0, c1 = GROUPS[b]
                ni = (c1 - c0) * NI_CH
                npt = (c1 - c0) * 128
                ag = ag_tiles.pop(b)
                # slot i = m*(19*16) + q*16 + p16 ; point jj = m*16 + p16
                ag4 = ag[:, 0:ni].rearrange(
                    "c (m q p) -> c m p q", m=npt // 16, q=KG, p=16
                )
                m_t = agpool.tile([128, npt], f32, tag="m")
                nc.vector.tensor_reduce(
                    out=m_t[:], in_=ag4, op=mybir.AluOpType.max,
                    axis=mybir.AxisListType.X,
                )
                ps = slice(128 * c0, 128 * c1)
                s_t = agpool.tile([128, npt], f32, tag="s")
                nc.vector.tensor_tensor(
                    out=s_t[:], in0=m_t[:], in1=g_t[:, ps], op=mybir.AluOpType.max
                )
                t_t = agpool.tile([128, npt], f32, tag="t")
                nc.vector.tensor_add(t_t[:], s_t[:], h_t[:, ps])
                o_t = agpool.tile([128, npt], f32, tag="o")
                nc.vector.tensor_scalar(
                    out=o_t[:], in0=t_t[:],
                    scalar1=hb_t[:], scalar2=0.0,
                    op0=mybir.AluOpType.add, op1=mybir.AluOpType.max,
                )
                out_tiles[b] = (o_t, ps)

            # out-writes are flushed one group late so a write stalled on its
            # producer never sits ahead of the next group's index loads in the
            # Sync engine stream
            def flush_out(b):
                o_t, ps = out_tiles.pop(b)
                nc.sync.dma_start(out_dram[:, ps], o_t[:])

            # Emission schedule: start gather g as soon as its chunks' top-k
            # is emitted; emit the finish (Vector reduce) one group later so a
            # reduce stalled on an in-flight gather sits behind as little of
            # the index-producing top-k stream as possible.
            next_start = 0
            for c in range(NCHUNK):
                emit_chunk(c)
                while next_start < len(GROUPS) and GROUPS[next_start][1] == c + 1:
                    emit_gather_start(next_start)
                    if next_start >= 1:
                        emit_gather_finish(next_start - 1)
                    if next_start >= 2:
                        flush_out(next_start - 2)
                    next_start += 1
            emit_gather_finish(len(GROUPS) - 1)
            flush_out(len(GROUPS) - 2)
            flush_out(len(GROUPS) - 1)

    nc.compile()
    return nc


def _fold_params(w1, b1, g1, be1, m1, v1, w2, b2, g2, be2, m2, v2):
    s1 = g1 / np.sqrt(v1 + EPS)
    sh1 = be1 - m1 * s1
    s2 = g2 / np.sqrt(v2 + EPS)
    sh2 = be2 - m2 * s2
    wg_xyz = (s1[None, :] * w1[:, 0:3].T).astype(np.float32)        # (3, 64)
    wh_xyz = (s1[None, :] * (w1[:, 3:6] - w1[:, 0:3]).T).astype(np.float32)
    wg_feat = (s2[None, :] * w2[:, 0:C].T).astype(np.float32)       # (128, 64)
    wh_feat = (s2[None, :] * (w2[:, C:2 * C] - w2[:, 0:C]).T).astype(np.float32)
    hb = np.concatenate([s1 * b1 + sh1, s2 * b2 + sh2]).astype(np.float32)[:, None]
    return wg_xyz, wg_feat, wh_xyz, wh_feat, hb


def kernel(xyz, features, w1, b1, g1, be1, m1, v1, w2, b2, g2, be2, m2, v2, k):
    global _compiled
    assert int(k) == K
    from concourse.bass_utils import run_bass_kernel_spmd

    if _compiled is None:
        _compiled = _build()
    nc = _compiled

    wg_xyz, wg_feat, wh_xyz, wh_feat, hb = _fold_params(
        np.asarray(w1), np.asarray(b1), np.asarray(g1), np.asarray(be1),
        np.asarray(m1), np.asarray(v1), np.asarray(w2), np.asarray(b2),
        np.asarray(g2), np.asarray(be2), np.asarray(m2), np.asarray(v2),
    )
    xyz = np.ascontiguousarray(np.asarray(xyz, dtype=np.float32))
    features = np.ascontiguousarray(np.asarray(features, dtype=np.float32))

    in_maps = []
    for bb in range(B):
        in_maps.append({
            "xyz": xyz[bb],
            "feat": features[bb],
            "wg_xyz": wg_xyz, "wg_feat": wg_feat,
            "wh_xyz": wh_xyz, "wh_feat": wh_feat,
            "hb": hb,
        })
    res = run_bass_kernel_spmd(nc, in_maps, list(range(B)))
    out = np.stack([res.results[bb]["out"] for bb in range(B)], axis=0)
    return out.astype(np.float32)


# revision 21
# speedup vs baseline: 1.0634x; 1.0634x over previous
"""DGCNN-style graph conv kernel for Trainium2 (8 NeuronCores, data-parallel over batch).

Reference computation (per sample):
  idx = knn(xyz, 20)                        # top-20 by -||xi-xj||^2, per point
  geo = relu(BN1(w1 @ [nb_xyz - xyz; xyz]))
  fea = relu(BN2(w2 @ [nb_feat - feat; feat]))
  out = max_k concat([geo, fea])            # (128, N)

Algebraic collapse used here (relu/max commute, BN scale > 0):
  out[c, n] = relu( max_k G[c, idx[n, k]] + H[c, n] + hb[c] )
  G = s * (Wa @ X)          (neighbor part, gathered)
  H = s * ((Wb - Wa) @ X)   (center part)
  hb = s * b + shift        (folded BN bias)
where for c < 64: Wa/Wb from w1, X = xyz; for c >= 64: from w2, X = feat.

Neighbor 0 is always the point itself (self-distance is the unique maximum of
-d^2), so only 19 indices per point are gathered; the self term G[:, n] is
folded in with a plain elementwise max.

Device pipeline per core (1 sample):
  1. D-chunk (128 rows x 2048) = -(dist^2) via one K=5 augmented fp32 matmul:
     lhsT = [xyz; xx; 1], rhs = [2*xyz; -1; -xx]
  2. top-20 per row: 3 rounds of (max8, max_index8, match_replace8) on PSUM
  3. indices 1..19 -> DRAM in a 16-wrapped layout, reloaded for ap_gather
  4. ap_gather columns of G (SBUF), tensor_reduce max over k; the reduce for
     gather group b is emitted AFTER the top-k of group b+1 so the Vector
     engine never stalls behind an in-flight gather.
"""
import numpy as np

B, N, C, K = 8, 2048, 128, 20
KG = K - 1           # 19 gathered neighbors (self handled separately)
H2 = C // 2          # 64
EPS = 1e-5
NEG = -3.0e38
NCHUNK = N // 128    # 16 topk chunks
# gather groups in chunks: small first groups so the (serial, dominant)
# GpSimd gather stream starts right after chunk 0's top-k
GROUPS = [(0, 1), (1, 2), (2, 4), (4, 8), (8, 12), (12, 16)]
NI_CH = 128 * KG     # 2432 indices per chunk
NI_MAX = 4 * NI_CH   # largest gather (4 chunks)

_compiled = None


def _build():
    import concourse.bass as bass
    import concourse.bacc as bacc
    import concourse.mybir as mybir
    import concourse.tile as tile
    from concourse import library_config

    f32 = mybir.dt.float32
    u16 = mybir.dt.uint16

    nc = bacc.Bacc("TRN2")
    xyz_in = nc.declare_dram_parameter("xyz", [3, N], f32, isOutput=False)
    feat_in = nc.declare_dram_parameter("feat", [C, N], f32, isOutput=False)
    wg_xyz_in = nc.declare_dram_parameter("wg_xyz", [3, H2], f32, isOutput=False)
    wg_feat_in = nc.declare_dram_parameter("wg_feat", [C, H2], f32, isOutput=False)
    wh_xyz_in = nc.declare_dram_parameter("wh_xyz", [3, H2], f32, isOutput=False)
    wh_feat_in = nc.declare_dram_parameter("wh_feat", [C, H2], f32, isOutput=False)
    hb_in = nc.declare_dram_parameter("hb", [C, 1], f32, isOutput=False)
    out_dram = nc.declare_dram_parameter("out", [C, N], f32, isOutput=True)

    # wrapped index scratch: row p16 (16 rows), col (ch*152 + ph*19 + q)
    idxw_dram = nc.dram_tensor("idxw_scratch", [16, N * KG // 16], u16)

    with tile.TileContext(nc) as tc:
        with (
            tc.tile_pool(name="const", bufs=1) as cpool,
            tc.tile_pool(name="work", bufs=2) as wpool,
            tc.tile_pool(name="ag", bufs=2) as agpool,
            tc.tile_pool(name="psum", bufs=2, space="PSUM") as ppool,
        ):
            nc.gpsimd.load_library(library_config.ap_gather)

            xyz_t = cpool.tile([3, N], f32)
            feat_t = cpool.tile([C, N], f32)
            wgx_t = cpool.tile([3, H2], f32)
            wgf_t = cpool.tile([C, H2], f32)
            whx_t = cpool.tile([3, H2], f32)
            whf_t = cpool.tile([C, H2], f32)
            hb_t = cpool.tile([C, 1], f32)
            nc.sync.dma_start(xyz_t[:], xyz_in[:])
            nc.sync.dma_start(feat_t[:], feat_in[:])
            nc.sync.dma_start(wgx_t[:], wg_xyz_in[:])
            nc.sync.dma_start(wgf_t[:], wg_feat_in[:])
            nc.sync.dma_start(whx_t[:], wh_xyz_in[:])
            nc.sync.dma_start(whf_t[:], wh_feat_in[:])
            nc.sync.dma_start(hb_t[:], hb_in[:])

            # ---- xx[n] = sum_d xyz[d,n]^2 ----
            sq_t = cpool.tile([3, N], f32)
            nc.vector.tensor_tensor(
                out=sq_t[:], in0=xyz_t[:], in1=xyz_t[:], op=mybir.AluOpType.mult
            )
            ones3_t = cpool.tile([3, 1], f32)
            nc.vector.memset(ones3_t[:], 1.0)
            xx_ps = ppool.tile([1, N], f32, space="PSUM", tag="d")
            for j in range(4):
                nc.tensor.matmul(
                    out=xx_ps[:, 512 * j:512 * (j + 1)],
                    lhsT=ones3_t[:],
                    rhs=sq_t[:, 512 * j:512 * (j + 1)],
                    start=True, stop=True,
                )
            xx_t = cpool.tile([1, N], f32)
            nc.scalar.copy(xx_t[:], xx_ps[:])

            # ---- lhs5 = [xyz; xx; 1], rhs5 = [2 xyz; -1; -xx] ----
            # compute-engine ops must start at quadrant-aligned partitions, so
            # rows 3/4 are placed with SBUF->SBUF DMAs instead.
            lhs5 = cpool.tile([5, N], f32)
            rhs5 = cpool.tile([5, N], f32)
            ones_row = cpool.tile([1, N], f32)
            neg1_row = cpool.tile([1, N], f32)
            nxx_t = cpool.tile([1, N], f32)
            nc.vector.memset(ones_row[:], 1.0)
            nc.vector.memset(neg1_row[:], -1.0)
            nc.vector.tensor_scalar_mul(nxx_t[:], xx_t[:], -1.0)
            nc.vector.tensor_copy(lhs5[0:3, :], xyz_t[:])
            nc.vector.tensor_scalar_mul(rhs5[0:3, :], xyz_t[:], 2.0)
            nc.sync.dma_start(lhs5[3:4, :], xx_t[:])
            nc.sync.dma_start(lhs5[4:5, :], ones_row[:])
            nc.sync.dma_start(rhs5[3:4, :], neg1_row[:])
            nc.sync.dma_start(rhs5[4:5, :], nxx_t[:])

            # ---- G, H (128, N) ----
            g_ps = ppool.tile([C, N], f32, space="PSUM", tag="d")
            for j in range(4):
                fs = slice(512 * j, 512 * (j + 1))
                nc.tensor.matmul(out=g_ps[0:H2, fs], lhsT=wgx_t[:], rhs=xyz_t[:, fs],
                                 start=True, stop=True)
            for j in range(4):
                fs = slice(512 * j, 512 * (j + 1))
                nc.tensor.matmul(out=g_ps[H2:C, fs], lhsT=wgf_t[:], rhs=feat_t[:, fs],
                                 start=True, stop=True)
            g_t = cpool.tile([C, N], f32)
            nc.scalar.copy(g_t[:], g_ps[:])

            h_ps = ppool.tile([C, N], f32, space="PSUM", tag="d")
            for j in range(4):
                fs = slice(512 * j, 512 * (j + 1))
                nc.tensor.matmul(out=h_ps[0:H2, fs], lhsT=whx_t[:], rhs=xyz_t[:, fs],
                                 start=True, stop=True)
            for j in range(4):
                fs = slice(512 * j, 512 * (j + 1))
                nc.tensor.matmul(out=h_ps[H2:C, fs], lhsT=whf_t[:], rhs=feat_t[:, fs],
                                 start=True, stop=True)
            h_t = cpool.tile([C, N], f32)
            nc.scalar.copy(h_t[:], h_ps[:])

            # wrapped idx write view: (16, NCHUNK*8*KG) -> [ch, ph, p16, q]
            idxw_w = idxw_dram[:].rearrange(
                "p (ch ph q) -> ch ph p q", ch=NCHUNK, ph=8, q=KG
            )

            # ---- per-chunk: D matmul + top-20 ----
            def emit_chunk(c):
                d_ps = ppool.tile([128, N], f32, space="PSUM", tag="d")
                for j in range(4):
                    fs = slice(512 * j, 512 * (j + 1))
                    nc.tensor.matmul(
                        out=d_ps[:, fs],
                        lhsT=lhs5[:, 128 * c:128 * (c + 1)],
                        rhs=rhs5[:, fs],
                        start=True, stop=True,
                    )
                d_sb = wpool.tile([128, N], f32, tag="dsb")
                nc.scalar.copy(d_sb[:], d_ps[:])
                vals = wpool.tile([128, 24], f32, tag="vals")
                idxs = wpool.tile([128, 24], u16, tag="idxs")
                for r in range(3):
                    v8 = vals[:, 8 * r:8 * (r + 1)]
                    i8 = idxs[:, 8 * r:8 * (r + 1)]
                    nc.vector.max(out=v8, in_=d_sb[:])
                    nc.vector.max_index(out=i8, in_max=v8, in_values=d_sb[:])
                    if r < 2:
                        nc.vector.match_replace(
                            out=d_sb[:], in_to_replace=v8, in_values=d_sb[:],
                            imm_value=NEG,
                        )
                # write top 1..19 indices (skip self at slot 0)
                nc.sync.dma_start(idxw_w[c], idxs[:, 1:K])

            # ---- gather start: idx reload + ap_gather (GpSimd + DMA only) ----
            ag_tiles = {}
            out_tiles = {}

            def emit_gather_start(b):
                c0, c1 = GROUPS[b]
                ni = (c1 - c0) * NI_CH
                idxw_t = agpool.tile([128, NI_MAX // 16], u16, tag="idxw")
                for g in range(8):
                    nc.sync.dma_start(
                        idxw_t[16 * g:16 * (g + 1), 0:ni // 16],
                        idxw_dram[:, (NI_CH // 16) * c0:(NI_CH // 16) * c1],
                    )
                ag = agpool.tile([128, NI_MAX], f32, tag="ag")
                nc.gpsimd.ap_gather(
                    out_ap=ag[:, 0:ni],
                    in_ap=g_t[:],
                    idxs_ap=idxw_t[:, 0:ni // 16].bitcast(mybir.dt.int16),
                    channels=128, num_elems=N, d=1, num_idxs=ni,
                )
                ag_tiles[b] = ag

            # ---- gather finish: reduce + self-max + bias + relu (Vector) ----
            def emit_gather_finish(b):
                c0, c1 = GROUPS[b]
                ni = (c1 - c0) * NI_CH
                npt = (c1 - c0) * 128
                ag = ag_tiles.pop(b)
                # slot i = m*(19*16) + q*16 + p16 ; point jj = m*16 + p16
                ag4 = ag[:, 0:ni].rearrange(
                    "c (m q p) -> c m p q", m=npt // 16, q=KG, p=16
                )
                m_t = agpool.tile([128, npt], f32, tag="m")
                nc.vector.tensor_reduce(
                    out=m_t[:], in_=ag4, op=mybir.AluOpType.max,
                    axis=mybir.AxisListType.X,
                )
                ps = slice(128 * c0, 128 * c1)
                s_t = agpool.tile([128, npt], f32, tag="s")
                nc.vector.tensor_tensor(
                    out=s_t[:], in0=m_t[:], in1=g_t[:, ps], op=mybir.AluOpType.max
                )
                t_t = agpool.tile([128, npt], f32, tag="t")
                nc.vector.tensor_add(t_t[:], s_t[:], h_t[:, ps])
                o_t = agpool.tile([128, npt], f32, tag="o")
                nc.vector.tensor_scalar(
                    out=o_t[:], in0=t_t[:],
                    scalar1=hb_t[:], scalar2=0.0,
                    op0=mybir.AluOpType.add, op1=mybir.AluOpType.max,
                )
                out_tiles[b] = (o_t, ps)

            # out-writes are flushed one group late so a write stalled on its
            # producer never sits ahead of the next group's index loads in the
            # Sync engine stream
            def flush_out(b):
                o_t, ps = out_tiles.pop(b)
                nc.sync.dma_start(out_dram[:, ps], o_t[:])

            # Emission schedule: start gather g as soon as its chunks' top-k
            # is emitted; emit the finish (Vector reduce) one group later so a
            # reduce stalled on an in-flight gather sits behind as little of
            # the index-producing top-k stream as possible.
            next_start = 0
            for c in range(NCHUNK):
                emit_chunk(c)
                while next_start < len(GROUPS) and GROUPS[next_start][1] == c + 1:
                    emit_gather_start(next_start)
                    if next_start >= 1:
                        emit_gather_finish(next_start - 1)
                    if next_start >= 2:
                        flush_out(next_start - 2)
                    next_start += 1
            emit_gather_finish(len(GROUPS) - 1)
            flush_out(len(GROUPS) - 2)
            flush_out(len(GROUPS) - 1)

    nc.compile()
    return nc


def _fold_params(w1, b1, g1, be1, m1, v1, w2, b2, g2, be2, m2, v2):
    s1 = g1 / np.sqrt(v1 + EPS)
    sh1 = be1 - m1 * s1
    s2 = g2 / np.sqrt(v2 + EPS)
    sh2 = be2 - m2 * s2
    wg_xyz = (s1[None, :] * w1[:, 0:3].T).astype(np.float32)        # (3, 64)
    wh_xyz = (s1[None, :] * (w1[:, 3:6] - w1[:, 0:3]).T).astype(np.float32)
    wg_feat = (s2[None, :] * w2[:, 0:C].T).astype(np.float32)       # (128, 64)
    wh_feat = (s2[None, :] * (w2[:, C:2 * C] - w2[:, 0:C]).T).astype(np.float32)
    hb = np.concatenate([s1 * b1 + sh1, s2 * b2 + sh2]).astype(np.float32)[:, None]
    return wg_xyz, wg_feat, wh_xyz, wh_feat, hb


def kernel(xyz, features, w1, b1, g1, be1, m1, v1, w2, b2, g2, be2, m2, v2, k):
    global _compiled
    assert int(k) == K
    from concourse.bass_utils import run_bass_kernel_spmd

    if _compiled is None:
        _compiled = _build()
    nc = _compiled

    wg_xyz, wg_feat, wh_xyz, wh_feat, hb = _fold_params(
        np.asarray(w1), np.asarray(b1), np.asarray(g1), np.asarray(be1),
        np.asarray(m1), np.asarray(v1), np.asarray(w2), np.asarray(b2),
        np.asarray(g2), np.asarray(be2), np.asarray(m2), np.asarray(v2),
    )
    xyz = np.ascontiguousarray(np.asarray(xyz, dtype=np.float32))
    features = np.ascontiguousarray(np.asarray(features, dtype=np.float32))

    in_maps = []
    for bb in range(B):
        in_maps.append({
            "xyz": xyz[bb],
            "feat": features[bb],
            "wg_xyz": wg_xyz, "wg_feat": wg_feat,
            "wh_xyz": wh_xyz, "wh_feat": wh_feat,
            "hb": hb,
        })
    res = run_bass_kernel_spmd(nc, in_maps, list(range(B)))
    out = np.stack([res.results[bb]["out"] for bb in range(B)], axis=0)
    return out.astype(np.float32)


# revision 29
# speedup vs baseline: 1.0788x; 1.0145x over previous
"""DGCNN-style graph conv kernel for Trainium2 (8 NeuronCores, data-parallel over batch).

Reference computation (per sample):
  idx = knn(xyz, 20)                        # top-20 by -||xi-xj||^2, per point
  geo = relu(BN1(w1 @ [nb_xyz - xyz; xyz]))
  fea = relu(BN2(w2 @ [nb_feat - feat; feat]))
  out = max_k concat([geo, fea])            # (128, N)

Algebraic collapse used here (relu/max commute, BN scale > 0):
  out[c, n] = relu( max_k G[c, idx[n, k]] + H[c, n] + hb[c] )
  G = s * (Wa @ X)          (neighbor part, gathered)
  H = s * ((Wb - Wa) @ X)   (center part)
  hb = s * b + shift        (folded BN bias)
where for c < 64: Wa/Wb from w1, X = xyz; for c >= 64: from w2, X = feat.

Neighbor 0 is always the point itself (self-distance is the unique maximum of
-d^2), so only 19 indices per point are gathered; the self term G[:, n] is
folded in with a plain elementwise max.

Device pipeline per core (1 sample):
  1. D-chunk (128 rows x 2048) = -(dist^2) via one K=5 augmented fp32 matmul:
     lhsT = [xyz; xx; 1], rhs = [2*xyz; -1; -xx]
  2. top-20 per row: 3 rounds of (max8, max_index8, match_replace8) on PSUM
  3. indices 1..19 -> DRAM in a 16-wrapped layout, reloaded for ap_gather
  4. ap_gather columns of G (SBUF), tensor_reduce max over k; the reduce for
     gather group b is emitted AFTER the top-k of group b+1 so the Vector
     engine never stalls behind an in-flight gather.
"""
import numpy as np

B, N, C, K = 8, 2048, 128, 20
KG = K - 1           # 19 gathered neighbors (self handled separately)
H2 = C // 2          # 64
EPS = 1e-5
NEG = -3.0e38
NCHUNK = N // 128    # 16 topk chunks
# gather groups in chunks: small first groups so the (serial, dominant)
# GpSimd gather stream starts right after chunk 0's top-k
GROUPS = [(0, 1), (1, 2), (2, 4), (4, 8), (8, 12), (12, 16)]
NI_CH = 128 * KG     # 2432 indices per chunk
NI_MAX = 4 * NI_CH   # largest gather (4 chunks)

_compiled = None


def _build():
    import concourse.bass as bass
    import concourse.bacc as bacc
    import concourse.mybir as mybir
    import concourse.tile as tile
    from concourse import library_config

    f32 = mybir.dt.float32
    u16 = mybir.dt.uint16

    nc = bacc.Bacc("TRN2")
    xyz_in = nc.declare_dram_parameter("xyz", [3, N], f32, isOutput=False)
    feat_in = nc.declare_dram_parameter("feat", [C, N], f32, isOutput=False)
    wg_xyz_in = nc.declare_dram_parameter("wg_xyz", [3, H2], f32, isOutput=False)
    wg_feat_in = nc.declare_dram_parameter("wg_feat", [C, H2], f32, isOutput=False)
    wh_xyz_in = nc.declare_dram_parameter("wh_xyz", [3, H2], f32, isOutput=False)
    wh_feat_in = nc.declare_dram_parameter("wh_feat", [C, H2], f32, isOutput=False)
    hb_in = nc.declare_dram_parameter("hb", [C, 1], f32, isOutput=False)
    out_dram = nc.declare_dram_parameter("out", [C, N], f32, isOutput=True)

    # wrapped index scratch: row p16 (16 rows), col (ch*152 + ph*19 + q)
    idxw_dram = nc.dram_tensor("idxw_scratch", [16, N * KG // 16], u16)

    with tile.TileContext(nc) as tc:
        with (
            tc.tile_pool(name="const", bufs=1) as cpool,
            tc.tile_pool(name="work", bufs=2) as wpool,
            tc.tile_pool(name="ag", bufs=2) as agpool,
            tc.tile_pool(name="psum", bufs=2, space="PSUM") as ppool,
        ):
            nc.gpsimd.load_library(library_config.ap_gather)

            xyz_t = cpool.tile([3, N], f32)
            feat_t = cpool.tile([C, N], f32)
            wgx_t = cpool.tile([3, H2], f32)
            wgf_t = cpool.tile([C, H2], f32)
            whx_t = cpool.tile([3, H2], f32)
            whf_t = cpool.tile([C, H2], f32)
            hb_t = cpool.tile([C, 1], f32)
            nc.sync.dma_start(xyz_t[:], xyz_in[:])
            nc.sync.dma_start(feat_t[:], feat_in[:])
            nc.sync.dma_start(wgx_t[:], wg_xyz_in[:])
            nc.sync.dma_start(wgf_t[:], wg_feat_in[:])
            nc.sync.dma_start(whx_t[:], wh_xyz_in[:])
            nc.sync.dma_start(whf_t[:], wh_feat_in[:])
            nc.sync.dma_start(hb_t[:], hb_in[:])

            # ---- xx[n] = sum_d xyz[d,n]^2 ----
            sq_t = cpool.tile([3, N], f32)
            nc.vector.tensor_tensor(
                out=sq_t[:], in0=xyz_t[:], in1=xyz_t[:], op=mybir.AluOpType.mult
            )
            ones3_t = cpool.tile([3, 1], f32)
            nc.vector.memset(ones3_t[:], 1.0)
            xx_ps = ppool.tile([1, N], f32, space="PSUM", tag="d")
            for j in range(4):
                nc.tensor.matmul(
                    out=xx_ps[:, 512 * j:512 * (j + 1)],
                    lhsT=ones3_t[:],
                    rhs=sq_t[:, 512 * j:512 * (j + 1)],
                    start=True, stop=True,
                )
            xx_t = cpool.tile([1, N], f32)
            nc.scalar.copy(xx_t[:], xx_ps[:])

            # ---- lhs5 = [xyz; xx; 1], rhs5 = [2 xyz; -1; -xx] ----
            # compute-engine ops must start at quadrant-aligned partitions, so
            # rows 3/4 are placed with SBUF->SBUF DMAs instead.
            lhs5 = cpool.tile([5, N], f32)
            rhs5 = cpool.tile([5, N], f32)
            ones_row = cpool.tile([1, N], f32)
            neg1_row = cpool.tile([1, N], f32)
            nxx_t = cpool.tile([1, N], f32)
            nc.vector.memset(ones_row[:], 1.0)
            nc.vector.memset(neg1_row[:], -1.0)
            nc.vector.tensor_scalar_mul(nxx_t[:], xx_t[:], -1.0)
            nc.vector.tensor_copy(lhs5[0:3, :], xyz_t[:])
            nc.vector.tensor_scalar_mul(rhs5[0:3, :], xyz_t[:], 2.0)
            nc.sync.dma_start(lhs5[3:4, :], xx_t[:])
            nc.sync.dma_start(lhs5[4:5, :], ones_row[:])
            nc.sync.dma_start(rhs5[3:4, :], neg1_row[:])
            nc.sync.dma_start(rhs5[4:5, :], nxx_t[:])

            # ---- G, H (128, N) ----
            g_ps = ppool.tile([C, N], f32, space="PSUM", tag="d")
            for j in range(4):
                fs = slice(512 * j, 512 * (j + 1))
                nc.tensor.matmul(out=g_ps[0:H2, fs], lhsT=wgx_t[:], rhs=xyz_t[:, fs],
                                 start=True, stop=True)
            for j in range(4):
                fs = slice(512 * j, 512 * (j + 1))
                nc.tensor.matmul(out=g_ps[H2:C, fs], lhsT=wgf_t[:], rhs=feat_t[:, fs],
                                 start=True, stop=True)
            g_t = cpool.tile([C, N], f32)
            nc.scalar.copy(g_t[:], g_ps[:])

            # H is not needed until the first gather-finish (~130us in), so its
            # matmuls + copy are emitted after chunk 0 / gather 0 (emit_h below)
            # to keep chunk 0's d_sb copy early in the Scalar stream.
            h_t = cpool.tile([C, N], f32)

            def emit_h():
                h_ps = ppool.tile([C, N], f32, space="PSUM", tag="d")
                for j in range(4):
                    fs = slice(512 * j, 512 * (j + 1))
                    nc.tensor.matmul(out=h_ps[0:H2, fs], lhsT=whx_t[:],
                                     rhs=xyz_t[:, fs], start=True, stop=True)
                for j in range(4):
                    fs = slice(512 * j, 512 * (j + 1))
                    nc.tensor.matmul(out=h_ps[H2:C, fs], lhsT=whf_t[:],
                                     rhs=feat_t[:, fs], start=True, stop=True)
                nc.scalar.copy(h_t[:], h_ps[:])

            # wrapped idx write view: (16, NCHUNK*8*KG) -> [ch, ph, p16, q]
            idxw_w = idxw_dram[:].rearrange(
                "p (ch ph q) -> ch ph p q", ch=NCHUNK, ph=8, q=KG
            )

            # ---- per-chunk: D matmul + top-20 ----
            def emit_chunk(c):
                d_ps = ppool.tile([128, N], f32, space="PSUM", tag="d")
                for j in range(4):
                    fs = slice(512 * j, 512 * (j + 1))
                    nc.tensor.matmul(
                        out=d_ps[:, fs],
                        lhsT=lhs5[:, 128 * c:128 * (c + 1)],
                        rhs=rhs5[:, fs],
                        start=True, stop=True,
                    )
                d_sb = wpool.tile([128, N], f32, tag="dsb")
                nc.scalar.copy(d_sb[:], d_ps[:])
                vals = wpool.tile([128, 24], f32, tag="vals")
                idxs = wpool.tile([128, 24], u16, tag="idxs")
                for r in range(3):
                    v8 = vals[:, 8 * r:8 * (r + 1)]
                    i8 = idxs[:, 8 * r:8 * (r + 1)]
                    nc.vector.max(out=v8, in_=d_sb[:])
                    nc.vector.max_index(out=i8, in_max=v8, in_values=d_sb[:])
                    if r < 2:
                        nc.vector.match_replace(
                            out=d_sb[:], in_to_replace=v8, in_values=d_sb[:],
                            imm_value=NEG,
                        )
                # write top 1..19 indices (skip self at slot 0)
                nc.sync.dma_start(idxw_w[c], idxs[:, 1:K])

            # ---- gather start: idx reload + ap_gather (GpSimd + DMA only) ----
            ag_tiles = {}
            out_tiles = {}

            def emit_gather_start(b):
                c0, c1 = GROUPS[b]
                ni = (c1 - c0) * NI_CH
                idxw_t = agpool.tile([128, NI_MAX // 16], u16, tag="idxw")
                for g in range(8):
                    nc.sync.dma_start(
                        idxw_t[16 * g:16 * (g + 1), 0:ni // 16],
                        idxw_dram[:, (NI_CH // 16) * c0:(NI_CH // 16) * c1],
                    )
                ag = agpool.tile([128, NI_MAX], f32, tag="ag")
                # the last group's gather is split so the bulk of its reduce
                # overlaps the final gather slice
                splits = [c0, c1 - 1, c1] if b == len(GROUPS) - 1 else [c0, c1]
                for s0, s1 in zip(splits, splits[1:]):
                    o0 = (s0 - c0) * NI_CH
                    o1 = (s1 - c0) * NI_CH
                    nc.gpsimd.ap_gather(
                        out_ap=ag[:, o0:o1],
                        in_ap=g_t[:],
                        idxs_ap=idxw_t[:, o0 // 16:o1 // 16].bitcast(
                            mybir.dt.int16),
                        channels=128, num_elems=N, d=1, num_idxs=o1 - o0,
                    )
                ag_tiles[b] = ag

            # ---- gather finish: reduce + self-max + bias + relu (Vector) ----
            def emit_gather_finish(b, cs=None, ce=None):
                c0, c1 = GROUPS[b]
                cs = c0 if cs is None else cs
                ce = c1 if ce is None else ce
                ni = (ce - cs) * NI_CH
                npt = (ce - cs) * 128
                ag = ag_tiles[b]
                if ce == c1:
                    ag_tiles.pop(b)
                o0 = (cs - c0) * NI_CH
                # slot i = m*(19*16) + q*16 + p16 ; point jj = m*16 + p16
                ag4 = ag[:, o0:o0 + ni].rearrange(
                    "c (m q p) -> c m p q", m=npt // 16, q=KG, p=16
                )
                m_t = agpool.tile([128, npt], f32, tag="m")
                nc.vector.tensor_reduce(
                    out=m_t[:], in_=ag4, op=mybir.AluOpType.max,
                    axis=mybir.AxisListType.X,
                )
                ps = slice(128 * cs, 128 * ce)
                s_t = agpool.tile([128, npt], f32, tag="s")
                nc.vector.tensor_tensor(
                    out=s_t[:], in0=m_t[:], in1=g_t[:, ps], op=mybir.AluOpType.max
                )
                t_t = agpool.tile([128, npt], f32, tag="t")
                nc.vector.tensor_add(t_t[:], s_t[:], h_t[:, ps])
                o_t = agpool.tile([128, npt], f32, tag="o")
                nc.vector.tensor_scalar(
                    out=o_t[:], in0=t_t[:],
                    scalar1=hb_t[:], scalar2=0.0,
                    op0=mybir.AluOpType.add, op1=mybir.AluOpType.max,
                )
                out_tiles[(b, cs)] = (o_t, ps)

            # out-writes are flushed one group late so a write stalled on its
            # producer never sits ahead of the next group's index loads in the
            # Sync engine stream
            def flush_out():
                for key in list(out_tiles):
                    o_t, ps = out_tiles.pop(key)
                    nc.sync.dma_start(out_dram[:, ps], o_t[:])

            # Emission schedule: start gather g as soon as its chunks' top-k
            # is emitted; emit the finish (Vector reduce) one group later so a
            # reduce stalled on an in-flight gather sits behind as little of
            # the index-producing top-k stream as possible.
            next_start = 0
            for c in range(NCHUNK):
                emit_chunk(c)
                while next_start < len(GROUPS) and GROUPS[next_start][1] == c + 1:
                    emit_gather_start(next_start)
                    if next_start == 0:
                        emit_h()
                    if next_start >= 1:
                        flush_out()
                        emit_gather_finish(next_start - 1)
                    next_start += 1
            lb = len(GROUPS) - 1
            lc0, lc1 = GROUPS[lb]
            flush_out()
            emit_gather_finish(lb, lc0, lc1 - 1)
            emit_gather_finish(lb, lc1 - 1, lc1)
            flush_out()

    nc.compile()
    return nc


def _fold_params(w1, b1, g1, be1, m1, v1, w2, b2, g2, be2, m2, v2):
    s1 = g1 / np.sqrt(v1 + EPS)
    sh1 = be1 - m1 * s1
    s2 = g2 / np.sqrt(v2 + EPS)
    sh2 = be2 - m2 * s2
    wg_xyz = (s1[None, :] * w1[:, 0:3].T).astype(np.float32)        # (3, 64)
    wh_xyz = (s1[None, :] * (w1[:, 3:6] - w1[:, 0:3]).T).astype(np.float32)
    wg_feat = (s2[None, :] * w2[:, 0:C].T).astype(np.float32)       # (128, 64)
    wh_feat = (s2[None, :] * (w2[:, C:2 * C] - w2[:, 0:C]).T).astype(np.float32)
    hb = np.concatenate([s1 * b1 + sh1, s2 * b2 + sh2]).astype(np.float32)[:, None]
    return wg_xyz, wg_feat, wh_xyz, wh_feat, hb


def kernel(xyz, features, w1, b1, g1, be1, m1, v1, w2, b2, g2, be2, m2, v2, k):
    global _compiled
    assert int(k) == K
    from concourse.bass_utils import run_bass_kernel_spmd

    if _compiled is None:
        _compiled = _build()
    nc = _compiled

    wg_xyz, wg_feat, wh_xyz, wh_feat, hb = _fold_params(
        np.asarray(w1), np.asarray(b1), np.asarray(g1), np.asarray(be1),
        np.asarray(m1), np.asarray(v1), np.asarray(w2), np.asarray(b2),
        np.asarray(g2), np.asarray(be2), np.asarray(m2), np.asarray(v2),
    )
    xyz = np.ascontiguousarray(np.asarray(xyz, dtype=np.float32))
    features = np.ascontiguousarray(np.asarray(features, dtype=np.float32))

    in_maps = []
    for bb in range(B):
        in_maps.append({
            "xyz": xyz[bb],
            "feat": features[bb],
            "wg_xyz": wg_xyz, "wg_feat": wg_feat,
            "wh_xyz": wh_xyz, "wh_feat": wh_feat,
            "hb": hb,
        })
    res = run_bass_kernel_spmd(nc, in_maps, list(range(B)))
    out = np.stack([res.results[bb]["out"] for bb in range(B)], axis=0)
    return out.astype(np.float32)


# revision 31
# speedup vs baseline: 1.0810x; 1.0021x over previous
"""DGCNN-style graph conv kernel for Trainium2 (8 NeuronCores, data-parallel over batch).

Reference computation (per sample):
  idx = knn(xyz, 20)                        # top-20 by -||xi-xj||^2, per point
  geo = relu(BN1(w1 @ [nb_xyz - xyz; xyz]))
  fea = relu(BN2(w2 @ [nb_feat - feat; feat]))
  out = max_k concat([geo, fea])            # (128, N)

Algebraic collapse used here (relu/max commute, BN scale > 0):
  out[c, n] = relu( max_k G[c, idx[n, k]] + H[c, n] + hb[c] )
  G = s * (Wa @ X)          (neighbor part, gathered)
  H = s * ((Wb - Wa) @ X)   (center part)
  hb = s * b + shift        (folded BN bias)
where for c < 64: Wa/Wb from w1, X = xyz; for c >= 64: from w2, X = feat.

Neighbor 0 is always the point itself (self-distance is the unique maximum of
-d^2), so only 19 indices per point are gathered; the self term G[:, n] is
folded in with a plain elementwise max.

Device pipeline per core (1 sample):
  1. D-chunk (128 rows x 2048) = -(dist^2) via one K=5 augmented fp32 matmul:
     lhsT = [xyz; xx; 1], rhs = [2*xyz; -1; -xx]
  2. top-20 per row: 3 rounds of (max8, max_index8, match_replace8) on PSUM
  3. indices 1..19 -> DRAM in a 16-wrapped layout, reloaded for ap_gather
  4. ap_gather columns of G (SBUF), tensor_reduce max over k; the reduce for
     gather group b is emitted AFTER the top-k of group b+1 so the Vector
     engine never stalls behind an in-flight gather.
"""
import numpy as np

B, N, C, K = 8, 2048, 128, 20
KG = K - 1           # 19 gathered neighbors (self handled separately)
H2 = C // 2          # 64
EPS = 1e-5
NEG = -3.0e38
NCHUNK = N // 128    # 16 topk chunks
# gather groups in chunks: small first groups so the (serial, dominant)
# GpSimd gather stream starts right after chunk 0's top-k
GROUPS = [(0, 1), (1, 2), (2, 4), (4, 8), (8, 12), (12, 16)]
NI_CH = 128 * KG     # 2432 indices per chunk
NI_MAX = 4 * NI_CH   # largest gather (4 chunks)

_compiled = None


def _build():
    import concourse.bass as bass
    import concourse.bacc as bacc
    import concourse.mybir as mybir
    import concourse.tile as tile
    from concourse import library_config

    f32 = mybir.dt.float32
    u16 = mybir.dt.uint16

    nc = bacc.Bacc("TRN2")
    xyz_in = nc.declare_dram_parameter("xyz", [3, N], f32, isOutput=False)
    feat_in = nc.declare_dram_parameter("feat", [C, N], f32, isOutput=False)
    wg_xyz_in = nc.declare_dram_parameter("wg_xyz", [3, H2], f32, isOutput=False)
    wg_feat_in = nc.declare_dram_parameter("wg_feat", [C, H2], f32, isOutput=False)
    wh_xyz_in = nc.declare_dram_parameter("wh_xyz", [3, H2], f32, isOutput=False)
    wh_feat_in = nc.declare_dram_parameter("wh_feat", [C, H2], f32, isOutput=False)
    hb_in = nc.declare_dram_parameter("hb", [C, 1], f32, isOutput=False)
    out_dram = nc.declare_dram_parameter("out", [C, N], f32, isOutput=True)

    # wrapped index scratch: row p16 (16 rows), col (ch*152 + ph*19 + q)
    idxw_dram = nc.dram_tensor("idxw_scratch", [16, N * KG // 16], u16)

    with tile.TileContext(nc) as tc:
        with (
            tc.tile_pool(name="const", bufs=1) as cpool,
            tc.tile_pool(name="work", bufs=2) as wpool,
            tc.tile_pool(name="ag", bufs=2) as agpool,
            tc.tile_pool(name="psum", bufs=2, space="PSUM") as ppool,
        ):
            nc.gpsimd.load_library(library_config.ap_gather)

            xyz_t = cpool.tile([3, N], f32)
            feat_t = cpool.tile([C, N], f32)
            wgx_t = cpool.tile([3, H2], f32)
            wgf_t = cpool.tile([C, H2], f32)
            whx_t = cpool.tile([3, H2], f32)
            whf_t = cpool.tile([C, H2], f32)
            hb_t = cpool.tile([C, 1], f32)
            nc.sync.dma_start(xyz_t[:], xyz_in[:])
            nc.sync.dma_start(feat_t[:], feat_in[:])
            nc.sync.dma_start(wgx_t[:], wg_xyz_in[:])
            nc.sync.dma_start(wgf_t[:], wg_feat_in[:])
            nc.sync.dma_start(whx_t[:], wh_xyz_in[:])
            nc.sync.dma_start(whf_t[:], wh_feat_in[:])
            nc.sync.dma_start(hb_t[:], hb_in[:])

            # ---- xx[n] = sum_d xyz[d,n]^2 ----
            sq_t = cpool.tile([3, N], f32)
            nc.vector.tensor_tensor(
                out=sq_t[:], in0=xyz_t[:], in1=xyz_t[:], op=mybir.AluOpType.mult
            )
            ones3_t = cpool.tile([3, 1], f32)
            nc.vector.memset(ones3_t[:], 1.0)
            xx_ps = ppool.tile([1, N], f32, space="PSUM", tag="d")
            for j in range(4):
                nc.tensor.matmul(
                    out=xx_ps[:, 512 * j:512 * (j + 1)],
                    lhsT=ones3_t[:],
                    rhs=sq_t[:, 512 * j:512 * (j + 1)],
                    start=True, stop=True,
                )
            xx_t = cpool.tile([1, N], f32)
            nc.scalar.copy(xx_t[:], xx_ps[:])

            # ---- lhs5 = [xyz; xx; 1], rhs5 = [2 xyz; -1; -xx] ----
            # compute-engine ops must start at quadrant-aligned partitions, so
            # rows 3/4 are placed with SBUF->SBUF DMAs instead.
            lhs5 = cpool.tile([5, N], f32)
            rhs5 = cpool.tile([5, N], f32)
            ones_row = cpool.tile([1, N], f32)
            neg1_row = cpool.tile([1, N], f32)
            nxx_t = cpool.tile([1, N], f32)
            nc.vector.memset(ones_row[:], 1.0)
            nc.vector.memset(neg1_row[:], -1.0)
            nc.vector.tensor_scalar_mul(nxx_t[:], xx_t[:], -1.0)
            nc.vector.tensor_copy(lhs5[0:3, :], xyz_t[:])
            nc.vector.tensor_scalar_mul(rhs5[0:3, :], xyz_t[:], 2.0)
            nc.sync.dma_start(lhs5[3:4, :], xx_t[:])
            nc.sync.dma_start(lhs5[4:5, :], ones_row[:])
            nc.sync.dma_start(rhs5[3:4, :], neg1_row[:])
            nc.sync.dma_start(rhs5[4:5, :], nxx_t[:])

            # ---- G, H (128, N) ----
            # G's copy only has to land before gather 0 (~65us in), so it is
            # emitted after chunk 0's D matmul to keep d_sb0's copy at the
            # front of the Scalar stream (emit_g below, called from the loop).
            g_t = cpool.tile([C, N], f32)

            def emit_g():
                g_ps = ppool.tile([C, N], f32, space="PSUM", tag="d")
                for j in range(4):
                    fs = slice(512 * j, 512 * (j + 1))
                    nc.tensor.matmul(out=g_ps[0:H2, fs], lhsT=wgx_t[:],
                                     rhs=xyz_t[:, fs], start=True, stop=True)
                for j in range(4):
                    fs = slice(512 * j, 512 * (j + 1))
                    nc.tensor.matmul(out=g_ps[H2:C, fs], lhsT=wgf_t[:],
                                     rhs=feat_t[:, fs], start=True, stop=True)
                nc.scalar.copy(g_t[:], g_ps[:])

            # H is not needed until the first gather-finish (~130us in), so its
            # matmuls + copy are emitted after chunk 0 / gather 0 (emit_h below)
            # to keep chunk 0's d_sb copy early in the Scalar stream.
            h_t = cpool.tile([C, N], f32)

            def emit_h():
                h_ps = ppool.tile([C, N], f32, space="PSUM", tag="d")
                for j in range(4):
                    fs = slice(512 * j, 512 * (j + 1))
                    nc.tensor.matmul(out=h_ps[0:H2, fs], lhsT=whx_t[:],
                                     rhs=xyz_t[:, fs], start=True, stop=True)
                for j in range(4):
                    fs = slice(512 * j, 512 * (j + 1))
                    nc.tensor.matmul(out=h_ps[H2:C, fs], lhsT=whf_t[:],
                                     rhs=feat_t[:, fs], start=True, stop=True)
                nc.scalar.copy(h_t[:], h_ps[:])

            # wrapped idx write view: (16, NCHUNK*8*KG) -> [ch, ph, p16, q]
            idxw_w = idxw_dram[:].rearrange(
                "p (ch ph q) -> ch ph p q", ch=NCHUNK, ph=8, q=KG
            )

            # ---- per-chunk: D matmul + top-20 ----
            def emit_chunk(c):
                d_ps = ppool.tile([128, N], f32, space="PSUM", tag="d")
                for j in range(4):
                    fs = slice(512 * j, 512 * (j + 1))
                    nc.tensor.matmul(
                        out=d_ps[:, fs],
                        lhsT=lhs5[:, 128 * c:128 * (c + 1)],
                        rhs=rhs5[:, fs],
                        start=True, stop=True,
                    )
                d_sb = wpool.tile([128, N], f32, tag="dsb")
                nc.scalar.copy(d_sb[:], d_ps[:])
                vals = wpool.tile([128, 24], f32, tag="vals")
                idxs = wpool.tile([128, 24], u16, tag="idxs")
                for r in range(3):
                    v8 = vals[:, 8 * r:8 * (r + 1)]
                    i8 = idxs[:, 8 * r:8 * (r + 1)]
                    nc.vector.max(out=v8, in_=d_sb[:])
                    nc.vector.max_index(out=i8, in_max=v8, in_values=d_sb[:])
                    if r < 2:
                        nc.vector.match_replace(
                            out=d_sb[:], in_to_replace=v8, in_values=d_sb[:],
                            imm_value=NEG,
                        )
                # write top 1..19 indices (skip self at slot 0)
                nc.sync.dma_start(idxw_w[c], idxs[:, 1:K])

            # ---- gather start: idx reload + ap_gather (GpSimd + DMA only) ----
            ag_tiles = {}
            out_tiles = {}

            def emit_gather_start(b):
                c0, c1 = GROUPS[b]
                ni = (c1 - c0) * NI_CH
                idxw_t = agpool.tile([128, NI_MAX // 16], u16, tag="idxw")
                for g in range(8):
                    nc.sync.dma_start(
                        idxw_t[16 * g:16 * (g + 1), 0:ni // 16],
                        idxw_dram[:, (NI_CH // 16) * c0:(NI_CH // 16) * c1],
                    )
                ag = agpool.tile([128, NI_MAX], f32, tag="ag")
                # the last group's gather is split so the bulk of its reduce
                # overlaps the final gather slice
                splits = [c0, c1 - 1, c1] if b == len(GROUPS) - 1 else [c0, c1]
                for s0, s1 in zip(splits, splits[1:]):
                    o0 = (s0 - c0) * NI_CH
                    o1 = (s1 - c0) * NI_CH
                    nc.gpsimd.ap_gather(
                        out_ap=ag[:, o0:o1],
                        in_ap=g_t[:],
                        idxs_ap=idxw_t[:, o0 // 16:o1 // 16].bitcast(
                            mybir.dt.int16),
                        channels=128, num_elems=N, d=1, num_idxs=o1 - o0,
                    )
                ag_tiles[b] = ag

            # ---- gather finish: reduce + self-max + bias + relu (Vector) ----
            def emit_gather_finish(b, cs=None, ce=None):
                c0, c1 = GROUPS[b]
                cs = c0 if cs is None else cs
                ce = c1 if ce is None else ce
                ni = (ce - cs) * NI_CH
                npt = (ce - cs) * 128
                ag = ag_tiles[b]
                if ce == c1:
                    ag_tiles.pop(b)
                o0 = (cs - c0) * NI_CH
                # slot i = m*(19*16) + q*16 + p16 ; point jj = m*16 + p16
                ag4 = ag[:, o0:o0 + ni].rearrange(
                    "c (m q p) -> c m p q", m=npt // 16, q=KG, p=16
                )
                m_t = agpool.tile([128, npt], f32, tag="m")
                nc.vector.tensor_reduce(
                    out=m_t[:], in_=ag4, op=mybir.AluOpType.max,
                    axis=mybir.AxisListType.X,
                )
                ps = slice(128 * cs, 128 * ce)
                s_t = agpool.tile([128, npt], f32, tag="s")
                nc.vector.tensor_tensor(
                    out=s_t[:], in0=m_t[:], in1=g_t[:, ps], op=mybir.AluOpType.max
                )
                t_t = agpool.tile([128, npt], f32, tag="t")
                nc.vector.tensor_add(t_t[:], s_t[:], h_t[:, ps])
                o_t = agpool.tile([128, npt], f32, tag="o")
                nc.vector.tensor_scalar(
                    out=o_t[:], in0=t_t[:],
                    scalar1=hb_t[:], scalar2=0.0,
                    op0=mybir.AluOpType.add, op1=mybir.AluOpType.max,
                )
                out_tiles[(b, cs)] = (o_t, ps)

            # out-writes are flushed one group late so a write stalled on its
            # producer never sits ahead of the next group's index loads in the
            # Sync engine stream
            def flush_out():
                for key in list(out_tiles):
                    o_t, ps = out_tiles.pop(key)
                    nc.sync.dma_start(out_dram[:, ps], o_t[:])

            # Emission schedule: start gather g as soon as its chunks' top-k
            # is emitted; emit the finish (Vector reduce) one group later so a
            # reduce stalled on an in-flight gather sits behind as little of
            # the index-producing top-k stream as possible.
            next_start = 0
            for c in range(NCHUNK):
                emit_chunk(c)
                if c == 0:
                    emit_g()
                while next_start < len(GROUPS) and GROUPS[next_start][1] == c + 1:
                    emit_gather_start(next_start)
                    if next_start == 0:
                        emit_h()
                    if next_start >= 1:
                        flush_out()
                        emit_gather_finish(next_start - 1)
                    next_start += 1
            lb = len(GROUPS) - 1
            lc0, lc1 = GROUPS[lb]
            flush_out()
            emit_gather_finish(lb, lc0, lc1 - 1)
            emit_gather_finish(lb, lc1 - 1, lc1)
            flush_out()

    nc.compile()
    return nc


def _fold_params(w1, b1, g1, be1, m1, v1, w2, b2, g2, be2, m2, v2):
    s1 = g1 / np.sqrt(v1 + EPS)
    sh1 = be1 - m1 * s1
    s2 = g2 / np.sqrt(v2 + EPS)
    sh2 = be2 - m2 * s2
    wg_xyz = (s1[None, :] * w1[:, 0:3].T).astype(np.float32)        # (3, 64)
    wh_xyz = (s1[None, :] * (w1[:, 3:6] - w1[:, 0:3]).T).astype(np.float32)
    wg_feat = (s2[None, :] * w2[:, 0:C].T).astype(np.float32)       # (128, 64)
    wh_feat = (s2[None, :] * (w2[:, C:2 * C] - w2[:, 0:C]).T).astype(np.float32)
    hb = np.concatenate([s1 * b1 + sh1, s2 * b2 + sh2]).astype(np.float32)[:, None]
    return wg_xyz, wg_feat, wh_xyz, wh_feat, hb


def kernel(xyz, features, w1, b1, g1, be1, m1, v1, w2, b2, g2, be2, m2, v2, k):
    global _compiled
    assert int(k) == K
    from concourse.bass_utils import run_bass_kernel_spmd

    if _compiled is None:
        _compiled = _build()
    nc = _compiled

    wg_xyz, wg_feat, wh_xyz, wh_feat, hb = _fold_params(
        np.asarray(w1), np.asarray(b1), np.asarray(g1), np.asarray(be1),
        np.asarray(m1), np.asarray(v1), np.asarray(w2), np.asarray(b2),
        np.asarray(g2), np.asarray(be2), np.asarray(m2), np.asarray(v2),
    )
    xyz = np.ascontiguousarray(np.asarray(xyz, dtype=np.float32))
    features = np.ascontiguousarray(np.asarray(features, dtype=np.float32))

    in_maps = []
    for bb in range(B):
        in_maps.append({
            "xyz": xyz[bb],
            "feat": features[bb],
            "wg_xyz": wg_xyz, "wg_feat": wg_feat,
            "wh_xyz": wh_xyz, "wh_feat": wh_feat,
            "hb": hb,
        })
    res = run_bass_kernel_spmd(nc, in_maps, list(range(B)))
    out = np.stack([res.results[bb]["out"] for bb in range(B)], axis=0)
    return out.astype(np.float32)


# revision 35
# speedup vs baseline: 1.0883x; 1.0068x over previous
"""DGCNN-style graph conv kernel for Trainium2 (8 NeuronCores, data-parallel over batch).

Reference computation (per sample):
  idx = knn(xyz, 20)                        # top-20 by -||xi-xj||^2, per point
  geo = relu(BN1(w1 @ [nb_xyz - xyz; xyz]))
  fea = relu(BN2(w2 @ [nb_feat - feat; feat]))
  out = max_k concat([geo, fea])            # (128, N)

Algebraic collapse used here (relu/max commute, BN scale > 0):
  out[c, n] = relu( max_k G[c, idx[n, k]] + H[c, n] + hb[c] )
  G = s * (Wa @ X)          (neighbor part, gathered)
  H = s * ((Wb - Wa) @ X)   (center part)
  hb = s * b + shift        (folded BN bias)
where for c < 64: Wa/Wb from w1, X = xyz; for c >= 64: from w2, X = feat.

Neighbor 0 is always the point itself (self-distance is the unique maximum of
-d^2), so only 19 indices per point are gathered; the self term G[:, n] is
folded in with a plain elementwise max.

Device pipeline per core (1 sample):
  1. D-chunk (128 rows x 2048) = -(dist^2) via one K=5 augmented fp32 matmul:
     lhsT = [xyz; xx; 1], rhs = [2*xyz; -1; -xx]
  2. top-20 per row: 3 rounds of (max8, max_index8, match_replace8) on PSUM
  3. indices 1..19 -> DRAM in a 16-wrapped layout, reloaded for ap_gather
  4. ap_gather columns of G (SBUF), tensor_reduce max over k; the reduce for
     gather group b is emitted AFTER the top-k of group b+1 so the Vector
     engine never stalls behind an in-flight gather.
"""
import numpy as np

B, N, C, K = 8, 2048, 128, 20
KG = K - 1           # 19 gathered neighbors (self handled separately)
H2 = C // 2          # 64
EPS = 1e-5
NEG = -3.0e38
NCHUNK = N // 128    # 16 topk chunks
# gather groups in chunks: small first groups so the (serial, dominant)
# GpSimd gather stream starts right after chunk 0's top-k
GROUPS = [(0, 1), (1, 2), (2, 4), (4, 8), (8, 12), (12, 16)]
NI_CH = 128 * KG     # 2432 indices per chunk
NI_MAX = 4 * NI_CH   # largest gather (4 chunks)

_compiled = None


def _build():
    import concourse.bass as bass
    import concourse.bacc as bacc
    import concourse.mybir as mybir
    import concourse.tile as tile
    from concourse import library_config

    f32 = mybir.dt.float32
    u16 = mybir.dt.uint16

    nc = bacc.Bacc("TRN2")
    xyz_in = nc.declare_dram_parameter("xyz", [3, N], f32, isOutput=False)
    feat_in = nc.declare_dram_parameter("feat", [C, N], f32, isOutput=False)
    wg_xyz_in = nc.declare_dram_parameter("wg_xyz", [3, H2], f32, isOutput=False)
    wg_feat_in = nc.declare_dram_parameter("wg_feat", [C, H2], f32, isOutput=False)
    wh_xyz_in = nc.declare_dram_parameter("wh_xyz", [3, H2], f32, isOutput=False)
    wh_feat_in = nc.declare_dram_parameter("wh_feat", [C, H2], f32, isOutput=False)
    hb_in = nc.declare_dram_parameter("hb", [C, 1], f32, isOutput=False)
    out_dram = nc.declare_dram_parameter("out", [C, N], f32, isOutput=True)

    # wrapped index scratch, 4x-replicated at write time so each gather needs
    # only two reload DMAs: row (g4 p16), col (ch*152 + ph*19 + q)
    idxw_dram = nc.dram_tensor("idxw_scratch", [64, N * KG // 16], u16)

    with tile.TileContext(nc) as tc:
        with (
            tc.tile_pool(name="const", bufs=1) as cpool,
            tc.tile_pool(name="work", bufs=2) as wpool,
            tc.tile_pool(name="ag", bufs=2) as agpool,
            tc.tile_pool(name="psum", bufs=2, space="PSUM") as ppool,
        ):
            nc.gpsimd.load_library(library_config.ap_gather)

            xyz_t = cpool.tile([3, N], f32)
            feat_t = cpool.tile([C, N], f32)
            wgx_t = cpool.tile([3, H2], f32)
            wgf_t = cpool.tile([C, H2], f32)
            whx_t = cpool.tile([3, H2], f32)
            whf_t = cpool.tile([C, H2], f32)
            hb_t = cpool.tile([C, 1], f32)
            nc.sync.dma_start(xyz_t[:], xyz_in[:])
            nc.sync.dma_start(feat_t[:], feat_in[:])
            nc.sync.dma_start(wgx_t[:], wg_xyz_in[:])
            nc.sync.dma_start(wgf_t[:], wg_feat_in[:])
            nc.sync.dma_start(whx_t[:], wh_xyz_in[:])
            nc.sync.dma_start(whf_t[:], wh_feat_in[:])
            nc.sync.dma_start(hb_t[:], hb_in[:])

            # ---- xx[n] = sum_d xyz[d,n]^2 ----
            sq_t = cpool.tile([3, N], f32)
            nc.vector.tensor_tensor(
                out=sq_t[:], in0=xyz_t[:], in1=xyz_t[:], op=mybir.AluOpType.mult
            )
            ones3_t = cpool.tile([3, 1], f32)
            nc.vector.memset(ones3_t[:], 1.0)
            xx_ps = ppool.tile([1, N], f32, space="PSUM", tag="d")
            for j in range(4):
                nc.tensor.matmul(
                    out=xx_ps[:, 512 * j:512 * (j + 1)],
                    lhsT=ones3_t[:],
                    rhs=sq_t[:, 512 * j:512 * (j + 1)],
                    start=True, stop=True,
                )
            xx_t = cpool.tile([1, N], f32)
            nc.scalar.copy(xx_t[:], xx_ps[:])

            # ---- lhs5 = [xyz; xx; 1], rhs5 = [2 xyz; -1; -xx] ----
            # compute-engine ops must start at quadrant-aligned partitions, so
            # rows 3/4 are placed with SBUF->SBUF DMAs instead.
            lhs5 = cpool.tile([5, N], f32)
            rhs5 = cpool.tile([5, N], f32)
            ones_row = cpool.tile([1, N], f32)
            neg1_row = cpool.tile([1, N], f32)
            nxx_t = cpool.tile([1, N], f32)
            nc.vector.memset(ones_row[:], 1.0)
            nc.vector.memset(neg1_row[:], -1.0)
            nc.vector.tensor_scalar_mul(nxx_t[:], xx_t[:], -1.0)
            nc.vector.tensor_copy(lhs5[0:3, :], xyz_t[:])
            nc.vector.tensor_scalar_mul(rhs5[0:3, :], xyz_t[:], 2.0)
            nc.sync.dma_start(lhs5[3:4, :], xx_t[:])
            nc.sync.dma_start(lhs5[4:5, :], ones_row[:])
            nc.sync.dma_start(rhs5[3:4, :], neg1_row[:])
            nc.sync.dma_start(rhs5[4:5, :], nxx_t[:])

            # ---- G, H (128, N) ----
            # G's copy only has to land before gather 0 (~65us in), so it is
            # emitted after chunk 0's D matmul to keep d_sb0's copy at the
            # front of the Scalar stream (emit_g below, called from the loop).
            g_t = cpool.tile([C, N], f32)

            def emit_g():
                g_ps = ppool.tile([C, N], f32, space="PSUM", tag="d")
                for j in range(4):
                    fs = slice(512 * j, 512 * (j + 1))
                    nc.tensor.matmul(out=g_ps[0:H2, fs], lhsT=wgx_t[:],
                                     rhs=xyz_t[:, fs], start=True, stop=True)
                for j in range(4):
                    fs = slice(512 * j, 512 * (j + 1))
                    nc.tensor.matmul(out=g_ps[H2:C, fs], lhsT=wgf_t[:],
                                     rhs=feat_t[:, fs], start=True, stop=True)
                nc.scalar.copy(g_t[:], g_ps[:])

            # H is not needed until the first gather-finish (~130us in), so its
            # matmuls + copy are emitted after chunk 0 / gather 0 (emit_h below)
            # to keep chunk 0's d_sb copy early in the Scalar stream.
            h_t = cpool.tile([C, N], f32)

            def emit_h():
                h_ps = ppool.tile([C, N], f32, space="PSUM", tag="d")
                for j in range(4):
                    fs = slice(512 * j, 512 * (j + 1))
                    nc.tensor.matmul(out=h_ps[0:H2, fs], lhsT=whx_t[:],
                                     rhs=xyz_t[:, fs], start=True, stop=True)
                for j in range(4):
                    fs = slice(512 * j, 512 * (j + 1))
                    nc.tensor.matmul(out=h_ps[H2:C, fs], lhsT=whf_t[:],
                                     rhs=feat_t[:, fs], start=True, stop=True)
                nc.scalar.copy(h_t[:], h_ps[:])

            # wrapped idx write view: (64, NCHUNK*8*KG) -> [g4, ch, ph, p16, q]
            idxw_w = idxw_dram[:].rearrange(
                "(g p) (ch ph q) -> g ch ph p q", g=4, ch=NCHUNK, ph=8, q=KG
            )

            # ---- per-chunk: D matmul + top-20 ----
            def emit_chunk(c):
                d_ps = ppool.tile([128, N], f32, space="PSUM", tag="d")
                for j in range(4):
                    fs = slice(512 * j, 512 * (j + 1))
                    nc.tensor.matmul(
                        out=d_ps[:, fs],
                        lhsT=lhs5[:, 128 * c:128 * (c + 1)],
                        rhs=rhs5[:, fs],
                        start=True, stop=True,
                    )
                d_sb = wpool.tile([128, N], f32, tag="dsb")
                nc.scalar.copy(d_sb[:], d_ps[:])
                vals = wpool.tile([128, 24], f32, tag="vals")
                idxs = wpool.tile([128, 24], u16, tag="idxs")
                for r in range(3):
                    v8 = vals[:, 8 * r:8 * (r + 1)]
                    i8 = idxs[:, 8 * r:8 * (r + 1)]
                    nc.vector.max(out=v8, in_=d_sb[:])
                    nc.vector.max_index(out=i8, in_max=v8, in_values=d_sb[:])
                    if r < 2:
                        nc.vector.match_replace(
                            out=d_sb[:], in_to_replace=v8, in_values=d_sb[:],
                            imm_value=NEG,
                        )
                # write top 1..19 indices (skip self at slot 0)
                for g in range(4):
                    nc.sync.dma_start(idxw_w[g, c], idxs[:, 1:K])

            # ---- gather start: idx reload + ap_gather (GpSimd + DMA only) ----
            ag_tiles = {}
            out_tiles = {}

            def emit_gather_start(b):
                c0, c1 = GROUPS[b]
                ni = (c1 - c0) * NI_CH
                idxw_t = agpool.tile([128, NI_MAX // 16], u16, tag="idxw")
                for g in range(2):
                    nc.sync.dma_start(
                        idxw_t[64 * g:64 * (g + 1), 0:ni // 16],
                        idxw_dram[:, (NI_CH // 16) * c0:(NI_CH // 16) * c1],
                    )
                ag = agpool.tile([128, NI_MAX], f32, tag="ag")
                # the last group's gather is split so the bulk of its reduce
                # overlaps the final gather slice
                splits = [c0, c1 - 1, c1] if b == len(GROUPS) - 1 else [c0, c1]
                for s0, s1 in zip(splits, splits[1:]):
                    o0 = (s0 - c0) * NI_CH
                    o1 = (s1 - c0) * NI_CH
                    nc.gpsimd.ap_gather(
                        out_ap=ag[:, o0:o1],
                        in_ap=g_t[:],
                        idxs_ap=idxw_t[:, o0 // 16:o1 // 16].bitcast(
                            mybir.dt.int16),
                        channels=128, num_elems=N, d=1, num_idxs=o1 - o0,
                    )
                ag_tiles[b] = ag

            # ---- gather finish: reduce + self-max + bias + relu (Vector) ----
            def emit_gather_finish(b, cs=None, ce=None):
                c0, c1 = GROUPS[b]
                cs = c0 if cs is None else cs
                ce = c1 if ce is None else ce
                ni = (ce - cs) * NI_CH
                npt = (ce - cs) * 128
                ag = ag_tiles[b]
                if ce == c1:
                    ag_tiles.pop(b)
                o0 = (cs - c0) * NI_CH
                # slot i = m*(19*16) + q*16 + p16 ; point jj = m*16 + p16
                ag4 = ag[:, o0:o0 + ni].rearrange(
                    "c (m q p) -> c m p q", m=npt // 16, q=KG, p=16
                )
                m_t = agpool.tile([128, npt], f32, tag="m")
                nc.vector.tensor_reduce(
                    out=m_t[:], in_=ag4, op=mybir.AluOpType.max,
                    axis=mybir.AxisListType.X,
                )
                ps = slice(128 * cs, 128 * ce)
                s_t = agpool.tile([128, npt], f32, tag="s")
                nc.vector.tensor_tensor(
                    out=s_t[:], in0=m_t[:], in1=g_t[:, ps], op=mybir.AluOpType.max
                )
                t_t = agpool.tile([128, npt], f32, tag="t")
                nc.vector.tensor_add(t_t[:], s_t[:], h_t[:, ps])
                o_t = agpool.tile([128, npt], f32, tag="o")
                nc.vector.tensor_scalar(
                    out=o_t[:], in0=t_t[:],
                    scalar1=hb_t[:], scalar2=0.0,
                    op0=mybir.AluOpType.add, op1=mybir.AluOpType.max,
                )
                out_tiles[(b, cs)] = (o_t, ps)

            # out-writes are flushed one group late so a write stalled on its
            # producer never sits ahead of the next group's index loads in the
            # Sync engine stream
            def flush_out():
                for key in list(out_tiles):
                    o_t, ps = out_tiles.pop(key)
                    nc.sync.dma_start(out_dram[:, ps], o_t[:])

            # Emission schedule: start gather g as soon as its chunks' top-k
            # is emitted; emit the finish (Vector reduce) one group later so a
            # reduce stalled on an in-flight gather sits behind as little of
            # the index-producing top-k stream as possible.
            next_start = 0
            for c in range(NCHUNK):
                emit_chunk(c)
                if c == 0:
                    emit_g()
                while next_start < len(GROUPS) and GROUPS[next_start][1] == c + 1:
                    emit_gather_start(next_start)
                    if next_start == 0:
                        emit_h()
                    if next_start >= 1:
                        flush_out()
                        emit_gather_finish(next_start - 1)
                    next_start += 1
            lb = len(GROUPS) - 1
            lc0, lc1 = GROUPS[lb]
            flush_out()
            emit_gather_finish(lb, lc0, lc1 - 1)
            emit_gather_finish(lb, lc1 - 1, lc1)
            flush_out()

    nc.compile()
    return nc


def _fold_params(w1, b1, g1, be1, m1, v1, w2, b2, g2, be2, m2, v2):
    s1 = g1 / np.sqrt(v1 + EPS)
    sh1 = be1 - m1 * s1
    s2 = g2 / np.sqrt(v2 + EPS)
    sh2 = be2 - m2 * s2
    wg_xyz = (s1[None, :] * w1[:, 0:3].T).astype(np.float32)        # (3, 64)
    wh_xyz = (s1[None, :] * (w1[:, 3:6] - w1[:, 0:3]).T).astype(np.float32)
    wg_feat = (s2[None, :] * w2[:, 0:C].T).astype(np.float32)       # (128, 64)
    wh_feat = (s2[None, :] * (w2[:, C:2 * C] - w2[:, 0:C]).T).astype(np.float32)
    hb = np.concatenate([s1 * b1 + sh1, s2 * b2 + sh2]).astype(np.float32)[:, None]
    return wg_xyz, wg_feat, wh_xyz, wh_feat, hb


def kernel(xyz, features, w1, b1, g1, be1, m1, v1, w2, b2, g2, be2, m2, v2, k):
    global _compiled
    assert int(k) == K
    from concourse.bass_utils import run_bass_kernel_spmd

    if _compiled is None:
        _compiled = _build()
    nc = _compiled

    wg_xyz, wg_feat, wh_xyz, wh_feat, hb = _fold_params(
        np.asarray(w1), np.asarray(b1), np.asarray(g1), np.asarray(be1),
        np.asarray(m1), np.asarray(v1), np.asarray(w2), np.asarray(b2),
        np.asarray(g2), np.asarray(be2), np.asarray(m2), np.asarray(v2),
    )
    xyz = np.ascontiguousarray(np.asarray(xyz, dtype=np.float32))
    features = np.ascontiguousarray(np.asarray(features, dtype=np.float32))

    in_maps = []
    for bb in range(B):
        in_maps.append({
            "xyz": xyz[bb],
            "feat": features[bb],
            "wg_xyz": wg_xyz, "wg_feat": wg_feat,
            "wh_xyz": wh_xyz, "wh_feat": wh_feat,
            "hb": hb,
        })
    res = run_bass_kernel_spmd(nc, in_maps, list(range(B)))
    out = np.stack([res.results[bb]["out"] for bb in range(B)], axis=0)
    return out.astype(np.float32)
